# revision 5
# baseline (speedup 1.0000x reference)
"""MoE MLP (top-2 routing, 8 experts) on 8 Trainium2 NeuronCores.

Strategy (expert-parallel, per the sharding hint): each core owns one
expert's weights. The router (a [8,1024] matmul + softmax + top-2 —
0.05% of total FLOPs) runs on the host, which doubles as the dispatch
step: tokens are gathered per selected expert and shipped to that
expert's core, replacing the all-to-all. Each core runs a fused
gelu-MLP Bass kernel over its routed tokens:

    yT = w ⊙ (W_out^T @ gelu(W_in^T @ xT + b_in) + b_out)

in a transposed layout (tokens along the free axis) so both matmuls
keep the *weights* stationary on the PE array and no on-chip
transposes are needed anywhere. BOTH weight matrices live resident in
SBUF (loaded once at startup on dedicated DMA queues, host-packed into
large contiguous segments), so after ~60us the only DMA traffic is the
token stream in and the result stream out. The host scatter-adds the
per-expert results back into the full [B,S,D] output.

Matmuls run in fp16 (same PE throughput as bf16 — 4x fp32 — but 8x
finer mantissa; measured end-to-end error vs the fp32 reference is
~4e-4 scale-relative).

Startup is latency-critical: the first matmul only needs the first
128-column block of W_in stripe 0 plus the first 128-row k-slice of
the x chunk, so those land as separate fine-grained tiles (per-tile
dependency tracking) on two parallel HWDGE queues, and the PE starts
~2-3us in rather than waiting ~20us for monolithic 1MB loads. No PE
warm-up loop: the HAM ramp throttle costs less than the warm-up did.
"""

import contextlib
import ctypes
import os
import sys
import types
from contextlib import ExitStack

import numpy as np

import concourse.bass as bass
import concourse.mybir as mybir
import concourse.tile as tile
from concourse import bacc
from concourse.bass_utils import run_bass_kernel_spmd


def _install_ntff_hook():
    """Provide antenv.axon_hooks (absent in this image) so BASS_TRACE=1
    can capture NTFF profiles through the axon PJRT .so. No-op if the
    module already exists or the .so/symbols are unavailable."""
    try:
        from antenv.axon_hooks import get_axon_ntff_profile_hook  # noqa: F401
        return
    except ImportError:
        pass
    so_path = "/opt/axon/libaxon_pjrt.so"
    if not os.path.exists(so_path):
        return
    try:
        lib = ctypes.CDLL(so_path)
    except OSError:
        return
    if not hasattr(lib, "axon_start_nrt_profile"):
        return
    lib.axon_start_nrt_profile.argtypes = [
        ctypes.POINTER(ctypes.c_int64), ctypes.c_size_t]
    lib.axon_start_nrt_profile.restype = ctypes.c_int64
    lib.axon_stop_nrt_profile.argtypes = [ctypes.c_char_p]
    lib.axon_stop_nrt_profile.restype = ctypes.c_int64

    @contextlib.contextmanager
    def _hook(output_dir, device_ids):
        import jax
        jax.devices()  # force PJRT init so the .so's client exists
        if device_ids:
            ids = (ctypes.c_int64 * len(device_ids))(*device_ids)
            rc = lib.axon_start_nrt_profile(ids, len(device_ids))
        else:
            rc = lib.axon_start_nrt_profile(None, 0)
        if rc != 0:
            raise RuntimeError(f"axon_start_nrt_profile rc={rc}")
        try:
            yield
        finally:
            n = lib.axon_stop_nrt_profile(str(output_dir).encode())
            print(f"ntff profile: {n} file(s) -> {output_dir}", file=sys.stderr)

    import antenv
    mod = types.ModuleType("antenv.axon_hooks")
    mod.get_axon_ntff_profile_hook = lambda: _hook
    mod.set_axon_ntff_profile_hook = lambda h: None
    sys.modules["antenv.axon_hooks"] = mod
    antenv.axon_hooks = mod

B, S, D, F, E = 4, 2048, 1024, 4096, 8
T = B * S
TOP_K = 2
NCORES = 8
P = 128
ND, NF = D // P, F // P  # 8, 32
NFO = F // 512           # 8 (512-wide stripes of F)

# test.py pokes these for profiling info
LAST_RESULT = None

_cache = {}


def _chunk_list(C):
    """Token chunks (PSUM free-dim <= 512, multiples of 128).

    Chunks below 256 run LDWEIGHTS-bound on the PE (weight load ~60ns
    vs a 53ns N=128 matmul), so a short tail is split off the previous
    512 chunk into two >=256 pieces instead.
    """
    chunks = [512] * (C // 512)
    rem = C % 512
    if rem:
        if rem < 256 and chunks:
            total = 512 + rem
            a = ((total // 2 + 127) // 128) * 128
            chunks[-1] = a
            chunks.append(total - a)
        else:
            chunks.append(rem)
    return chunks


def _build_bass(C):
    dt = mybir.dt
    io_dt = dt.float16
    nc = bacc.Bacc("TRN2", target_bir_lowering=False, debug=False)

    xT = nc.dram_tensor("xT", [D, C], io_dt, kind="ExternalInput")
    # W_in host-packed for big contiguous DMA segments:
    #   win0: stripe fo=0 in four 128-col j-blocks, layout [p][j][dn][128]
    #   win:  stripes fo=1..7, layout [p][fo-1][dn][512]
    win0 = nc.dram_tensor("win0", [P, 4 * ND * 128], io_dt, kind="ExternalInput")
    win = nc.dram_tensor("win", [P, (NFO - 1) * ND * 512], io_dt,
                         kind="ExternalInput")
    wout = nc.dram_tensor("wout", [F, D], io_dt, kind="ExternalInput")
    bin_ = nc.dram_tensor("bin", [F], dt.float32, kind="ExternalInput")
    bout = nc.dram_tensor("bout", [D], dt.float32, kind="ExternalInput")
    wcomb = nc.dram_tensor("wcomb", [P, C], dt.float32, kind="ExternalInput")
    yT = nc.dram_tensor("yT", [D, C], dt.float32, kind="ExternalOutput")

    xT_r = xT.ap().rearrange("(dn p) c -> p dn c", p=P)
    win0_r = win0.ap().rearrange("p (j dn f) -> p j dn f", j=4, f=128)
    win_r = win.ap().rearrange("p (fo dn f) -> p fo dn f", fo=NFO - 1, f=512)
    wout_r = wout.ap().rearrange("(fn p) d -> p fn d", p=P)
    yT_r = yT.ap().rearrange("(dn p) c -> p dn c", p=P)

    chunks = _chunk_list(C)
    ck0 = chunks[0]

    with tile.TileContext(nc) as tc, ExitStack() as ctx:
        consts = ctx.enter_context(tc.tile_pool(name="consts", bufs=1))
        xpool = ctx.enter_context(tc.tile_pool(name="x", bufs=2))
        wrespool = ctx.enter_context(tc.tile_pool(name="wres", bufs=1))
        woutpool = ctx.enter_context(tc.tile_pool(name="wout", bufs=1))
        hpool = ctx.enter_context(tc.tile_pool(name="h", bufs=1))
        ypool = ctx.enter_context(tc.tile_pool(name="y", bufs=3))
        psum_h = ctx.enter_context(tc.tile_pool(name="ph", bufs=4, space="PSUM"))
        psum_y = ctx.enter_context(tc.tile_pool(name="py", bufs=2, space="PSUM"))

        # ---- resident weights + startup DMA schedule ------------------
        # TRN2 has two HWDGE trigger queues (sync/SP and scalar/Act) at
        # ~150 GB/s each plus the slow gpsimd SWDGE; per-queue order is
        # emission order. Queue S (sync): b_in (the first gelu needs it
        # ~5us in), W_in stripe-0 j-blocks (the first matmuls), then
        # even W_in stripes and even W_out stripes. Queue A (scalar):
        # x chunk-0 in per-k-slice tiles (the PE consumes them in
        # arrival order), then odd W_in/W_out stripes, then the token
        # chunks + y output. Stripe 1 is split across both queues so it
        # lands before its phase-A slot (~12us). Each W_in stripe k
        # arrives ~6.7us into a ~6.8us consumption slot; W_out
        # completes ~57us, just before phase B first needs it (~64us).
        # Queue G (gpsimd): small phase-B constants (needed ~65us).
        bin_t = consts.tile([P, NF], dt.float32)
        nc.sync.dma_start(bin_t[:], bin_.ap().rearrange("(fo fi) -> fi fo", fi=P))

        win0_t = []
        for j in range(4):
            t = wrespool.tile([P, ND, 128], io_dt, name=f"win0j{j}")
            nc.sync.dma_start(t[:], win0_r[:, j, :, :])
            win0_t.append(t)

        x0_t = []
        for dn in range(ND):
            t = xpool.tile([P, ck0], io_dt, name=f"x0dn{dn}")
            nc.scalar.dma_start(t[:], xT_r[:, dn, 0:ck0])
            x0_t.append(t)

        win_t = [None]
        for fo in range(1, NFO):
            t = wrespool.tile([P, ND, 512], io_dt, name=f"wfo{fo}")
            if fo == 1:
                nc.sync.dma_start(t[:, :4, :], win_r[:, 0, 0:4, :])
                nc.scalar.dma_start(t[:, 4:, :], win_r[:, 0, 4:, :])
            elif fo % 2 == 0:
                nc.sync.dma_start(t[:], win_r[:, fo - 1, :, :])
            else:
                nc.scalar.dma_start(t[:], win_r[:, fo - 1, :, :])
            win_t.append(t)

        wout_tiles = []
        for fo in range(NFO):
            t = woutpool.tile([P, 4, D], io_dt, name=f"wout{fo}")
            eng = nc.sync if fo % 2 == 0 else nc.scalar
            eng.dma_start(t[:], wout_r[:, fo * 4:(fo + 1) * 4, :])
            wout_tiles.append(t)

        bout_t = consts.tile([P, ND], dt.float32)
        nc.gpsimd.dma_start(bout_t[:],
                            bout.ap().rearrange("(do di) -> di do", di=P))
        w_t = consts.tile([P, C], dt.float32)
        nc.gpsimd.dma_start(w_t[:], wcomb.ap())

        def win_ap(fo, j, dn):
            if fo == 0:
                return win0_t[j][:, dn, :]
            return win_t[fo][:, dn, j * P:(j + 1) * P]

        # ---- main loop ------------------------------------------------
        off = 0
        for ci, ck in enumerate(chunks):
            csl = slice(off, off + ck)
            last = ci == len(chunks) - 1
            if ci > 0:
                x_t = xpool.tile([P, ND, ck], io_dt, tag="x")
                nc.scalar.dma_start(x_t[:], xT_r[:, :, csl])

            # ---- phase A: h = gelu(W_in^T @ x + b_in), laid out [f, tok]
            h_t = hpool.tile([P, NF, ck], io_dt, tag="h")
            for fo in range(NFO):
                for j in range(4):
                    fc = fo * 4 + j
                    ph = psum_h.tile([P, ck], dt.float32, tag="ph")
                    for dn in range(ND):
                        nc.tensor.matmul(
                            ph[:],
                            win_ap(fo, j, dn),
                            x0_t[dn][:] if ci == 0 else x_t[:, dn, :],
                            start=(dn == 0),
                            stop=(dn == ND - 1),
                        )
                    nc.scalar.activation(
                        h_t[:, fc, :], ph[:],
                        mybir.ActivationFunctionType.Gelu,
                        bias=bin_t[:, fc:fc + 1],
                    )

            # ---- phase B: y = w * (W_out^T @ h + b_out), laid out [d, tok]
            for dn in range(ND):
                py = psum_y.tile([P, ck], dt.float32, tag="py")
                for fc in range(NF):
                    nc.tensor.matmul(
                        py[:],
                        wout_tiles[fc // 4][:, fc % 4, dn * P:(dn + 1) * P],
                        h_t[:, fc, :],
                        start=(fc == 0),
                        stop=(fc == NF - 1),
                    )
                y_t = ypool.tile([P, ck], dt.float32, tag="y")
                # one DVE op: (psum + b_out) * w — keeps ScalarE on
                # gelu only (no ACT table switching per chunk)
                nc.vector.scalar_tensor_tensor(
                    y_t[:], py[:], bout_t[:, dn:dn + 1], w_t[:, csl],
                    op0=mybir.AluOpType.add, op1=mybir.AluOpType.mult,
                )
                # steady state keeps y on the scalar queue (idle after
                # startup); the final chunk alternates queues so the
                # output drain at end-of-kernel runs in parallel.
                if last:
                    eng = (nc.scalar, nc.sync)[dn % 2]
                else:
                    eng = nc.scalar
                eng.dma_start(yT_r[:, dn, csl], y_t[:])
            off += ck

    nc.compile()
    return nc


def _get_nc(C):
    if C not in _cache:
        _cache[C] = _build_bass(C)
    return _cache[C]


def _route(x, W_router):
    """Host-side router: top-2 selection + renormalized weights (fp64).

    Matches jax.lax.top_k on softmax(logits): softmax is monotone so
    top-2 of logits is identical, with ties broken toward lower index
    (argsort stable on -logits).
    """
    lg = x.astype(np.float64) @ W_router.T.astype(np.float64)
    top2 = np.argsort(-lg, axis=1, kind="stable")[:, :TOP_K]
    l1 = np.take_along_axis(lg, top2[:, 0:1], 1)
    l2 = np.take_along_axis(lg, top2[:, 1:2], 1)
    e2 = np.exp(l2 - l1)
    w1 = (1.0 / (1.0 + e2)).astype(np.float32)
    w2 = (e2 / (1.0 + e2)).astype(np.float32)
    return top2, np.concatenate([w1, w2], axis=1)


def _pack_win(W):
    """[D, F] fp16 -> (win0 [P, 4*ND*128], win [P, 7*ND*512]).

    win0 is stripe fo=0 as [p][j][dn][128] (four 128-col j-blocks, the
    unit the first matmuls consume); win is stripes 1..7 as
    [p][fo-1][dn][512]. Both give the DMA large contiguous
    per-partition segments.
    """
    A = W.reshape(ND, P, NFO, 512).transpose(1, 2, 0, 3)  # [p, fo, dn, f]
    w0 = A[:, 0].reshape(P, ND, 4, 128).transpose(0, 2, 1, 3)  # [p, j, dn, f]
    win0 = np.ascontiguousarray(w0).reshape(P, 4 * ND * 128)
    win = np.ascontiguousarray(A[:, 1:]).reshape(P, (NFO - 1) * ND * 512)
    return win0, win


def kernel(residual, W_router, W_in, b_in, W_out, b_out):
    global LAST_RESULT

    x = np.ascontiguousarray(np.asarray(residual, dtype=np.float32).reshape(T, D))
    W_in = np.asarray(W_in, dtype=np.float32)
    W_out = np.asarray(W_out, dtype=np.float32)
    b_in = np.asarray(b_in, dtype=np.float32)
    b_out = np.asarray(b_out, dtype=np.float32)

    top2, wts = _route(x, np.asarray(W_router, dtype=np.float32))

    idxs, ws = [], []
    for e in range(E):
        sel0 = top2[:, 0] == e
        sel1 = top2[:, 1] == e
        idx = np.concatenate([np.where(sel0)[0], np.where(sel1)[0]])
        w = np.concatenate([wts[sel0, 0], wts[sel1, 1]])
        idxs.append(idx)
        ws.append(w)

    C = max(len(i) for i in idxs)
    C = ((C + P - 1) // P) * P
    nc = _get_nc(C)

    xt = np.ascontiguousarray(x.T)  # [D, T]
    in_maps = []
    for e in range(E):
        cnt = len(idxs[e])
        xT_e = np.zeros((D, C), dtype=np.float16)
        xT_e[:, :cnt] = xt[:, idxs[e]]
        wc_e = np.zeros((P, C), dtype=np.float32)
        wc_e[:, :cnt] = ws[e][None, :]
        win0_e, win_e = _pack_win(np.asarray(W_in[e], dtype=np.float16))
        in_maps.append({
            "xT": xT_e,
            "win0": win0_e,
            "win": win_e,
            "wout": np.ascontiguousarray(W_out[e], dtype=np.float16),
            "bin": b_in[e],
            "bout": b_out[e],
            "wcomb": wc_e,
        })

    if os.environ.get("BASS_TRACE"):
        _install_ntff_hook()
    LAST_RESULT = run_bass_kernel_spmd(nc, in_maps, list(range(NCORES)))

    y = np.zeros((T, D), dtype=np.float32)
    for e in range(E):
        cnt = len(idxs[e])
        y[idxs[e]] += LAST_RESULT.results[e]["yT"][:, :cnt].T
    return y.reshape(B, S, D)
